# revision 1
# baseline (speedup 1.0000x reference)
"""CrossNetMix (DCN-V2 MoE cross-network) Trainium2 Bass kernel.

Math (per layer i, reference):
    v    = tanh(x_l @ V[i])      per expert      (B, E, R)
    c    = tanh(v @ C[i].T)      per expert      (B, E, R)
    u    = c @ U[i].T            per expert      (B, E, D)
    gate = softmax(x_l @ G.T)                    (B, E)
    x_l  = sum_e gate_e * x0 * (u_e + bias[i]) + x_l

Since softmax gates sum to 1 the update is
    x_{l+1} = x0 * (sum_e gate_e u_e + bias[i]) + x_l
and with S_0 = 1, x_l = x0 * S_l where
    S_{l+1} = S_l + umix_l + bias[i],   umix = U_arr^T (gate256 * c)
so the residual/bias fold into cheap per-chunk elementwise ops.

Device layout: features on partitions, tokens on the free dim.  The host
pre-transposes each core's x slice to (D, Bc) so every DMA is contiguous
and the whole matmul chain (V -> C -> U) stays feature-major with zero
on-device transposes.  Gate softmax over the E=4 partition dim is done
with tiny matmuls (ones(4,4) broadcast-sum, selector broadcast 4->256).

All matmul operand tiles are float32r-typed (full PE rate; the BIR
verifier requires fp32r producers for fp32r matmuls).  The S recurrence
runs through the PSUM accumulator: the U matmul leaves umix in PSUM, an
identity matmul accumulates S_l on top, and one DVE scalar_tensor_tensor
materializes x_{l+1} = (psum + bias) * x0 straight from PSUM.  ACT
copies S_{l+1} = psum + bias to SBUF (fused per-partition bias) only
when the next layer's identity matmul needs it.
"""

import numpy as np

import concourse.bacc as bacc
import concourse.bass as bass
import concourse.mybir as mybir
import concourse.tile as tile
from concourse.bass_utils import run_bass_kernel_spmd

# Problem constants (hardcoded per contract).
B, D, LAYERS, E, R = 16384, 1024, 3, 4, 64
ER = E * R                  # 256
NCORES = 8
BC = B // NCORES            # 2048 tokens per core
NB = 512                    # token block = PSUM bank width (fp32)
KC = D // 128               # 8 feature chunks
F32 = mybir.dt.float32
F32R = mybir.dt.float32r

AF = mybir.ActivationFunctionType
OP = mybir.AluOpType


def _emit(tc, outT, xT, w1, gt, cw, ua, biasP, sel, onesE, eye, n_blocks):
    nc = tc.nc
    from contextlib import ExitStack

    with ExitStack() as ctx:
        consts = ctx.enter_context(tc.tile_pool(name="consts", bufs=1))
        xin = ctx.enter_context(tc.tile_pool(name="xin", bufs=2))
        work = ctx.enter_context(tc.tile_pool(name="work", bufs=2))
        pp = ctx.enter_context(tc.tile_pool(name="pp", bufs=2, space="PSUM"))

        # ---- resident weights ----
        w1_sb = []          # [L][KC] tiles (128, ER): lhsT for V matmul
        ua_sb = []          # [L][2] tiles (128, D): lhsT for U matmul
        cw_sb = []          # [L][2] tiles (128, 128): block-diag C^T
        bias_sb = []        # [L] tiles (128, KC): bias column per d-chunk
        for i in range(LAYERS):
            per_k = []
            for k in range(KC):
                t = consts.tile([128, ER], F32R, name=f"w1_{i}_{k}")
                nc.sync.dma_start(out=t, in_=w1[i, k * 128:(k + 1) * 128, :])
                per_k.append(t)
            w1_sb.append(per_k)

            per_kc = []
            for kc in range(2):
                t = consts.tile([128, D], F32R, name=f"ua_{i}_{kc}")
                nc.sync.dma_start(out=t, in_=ua[i, kc * 128:(kc + 1) * 128, :])
                per_kc.append(t)
            ua_sb.append(per_kc)

            per_j = []
            for j in range(2):
                t = consts.tile([128, 128], F32R, name=f"cw_{i}_{j}")
                nc.sync.dma_start(out=t, in_=cw[i, j])
                per_j.append(t)
            cw_sb.append(per_j)

            t = consts.tile([128, KC], F32, name=f"bias_{i}")
            nc.sync.dma_start(out=t, in_=biasP[i].rearrange("(m p) -> p m", p=128))
            bias_sb.append(t)

        gt_sb = []
        for k in range(KC):
            t = consts.tile([128, E], F32R, name=f"gt_{k}")
            nc.sync.dma_start(out=t, in_=gt[k * 128:(k + 1) * 128, :])
            gt_sb.append(t)

        sel_sb = consts.tile([E, ER], F32R, name="sel")
        nc.sync.dma_start(out=sel_sb, in_=sel)
        onesE_sb = consts.tile([E, E], F32R, name="onesE")
        nc.sync.dma_start(out=onesE_sb, in_=onesE)
        eye_sb = consts.tile([128, 128], F32R, name="eye")
        nc.sync.dma_start(out=eye_sb, in_=eye)

        xT_r = xT.rearrange("(k p) t -> p k t", p=128)
        outT_r = outT.rearrange("(m p) t -> p m t", p=128)

        # ---- token-block loop ----
        for b in range(n_blocks):
            x0 = xin.tile([128, KC, NB], F32R, tag="x0", name=f"x0_{b}")
            for k in range(KC):
                nc.sync.dma_start(out=x0[:, k, :],
                                  in_=xT_r[:, k, b * NB:(b + 1) * NB])

            S_prev = None
            xl = x0  # layer 0 input is x0 itself (S_0 = 1)
            for l in range(LAYERS):
                # gate logits: (E, NB) psum, accumulate over feature chunks
                glog = pp.tile([E, NB], F32, tag="small", bufs=1, name=f"glog{b}_{l}")
                for k in range(KC):
                    nc.tensor.matmul(glog, (gt_sb[k]), (xl[:, k, :]),
                                     start=(k == 0), stop=(k == KC - 1))

                # v = tanh(W1^T x): two 128-row er-chunks
                vps = [pp.tile([128, NB], F32, tag="vps", name=f"vps{b}_{l}_{j}") for j in range(2)]
                for j in range(2):
                    for k in range(KC):
                        nc.tensor.matmul(
                            vps[j],
                            (w1_sb[l][k][:, j * 128:(j + 1) * 128]),
                            (xl[:, k, :]),
                            start=(k == 0), stop=(k == KC - 1))
                v_sb = [work.tile([128, NB], F32R, tag=f"vsb{j}", name=f"vsb{b}_{l}_{j}") for j in range(2)]
                for j in range(2):
                    nc.scalar.activation(v_sb[j], vps[j], AF.Tanh)

                # softmax over E=4 partitions via matmul broadcast-sum
                expg = work.tile([E, NB], F32R, tag="expg", name=f"expg{b}_{l}")
                nc.scalar.activation(expg, glog, AF.Exp)
                sumb = pp.tile([E, NB], F32, tag="small", bufs=1, name=f"sumb{b}_{l}")
                nc.tensor.matmul(sumb, (onesE_sb), (expg), start=True, stop=True)
                recip = work.tile([E, NB], F32, tag="recip", name=f"recip{b}_{l}")
                nc.vector.reciprocal(recip, sumb)
                gate = work.tile([E, NB], F32R, tag="gate", name=f"gate{b}_{l}")
                nc.vector.tensor_mul(gate, expg.bitcast(F32), recip)

                # c = tanh(blockdiag(C^T) v)
                cps = [pp.tile([128, NB], F32, tag="mid", name=f"cps{b}_{l}_{j}") for j in range(2)]
                for j in range(2):
                    nc.tensor.matmul(cps[j], (cw_sb[l][j]), (v_sb[j]),
                                     start=True, stop=True)
                c_sb = [work.tile([128, NB], F32R, tag=f"csb{j}", name=f"csb{b}_{l}_{j}") for j in range(2)]
                for j in range(2):
                    nc.scalar.activation(c_sb[j], cps[j], AF.Tanh)

                # broadcast gate (E, NB) -> (ER, NB) with the selector matmul
                gps = [pp.tile([128, NB], F32, tag="mid", name=f"gps{b}_{l}_{j}") for j in range(2)]
                for j in range(2):
                    nc.tensor.matmul(gps[j], (sel_sb[:, j * 128:(j + 1) * 128]),
                                     (gate), start=True, stop=True)
                cg = [work.tile([128, NB], F32R, tag=f"cg{j}", name=f"cg{b}_{l}_{j}") for j in range(2)]
                for j in range(2):
                    nc.vector.tensor_mul(cg[j], c_sb[j].bitcast(F32), gps[j])

                # umix per d-chunk + S update.
                # S_{l+1} = umix + bias_l + S_l: S_l joins via an identity
                # matmul accumulating into the psum (PE), bias via the ACT
                # fused per-partition bias on the psum->SBUF copy, so DVE
                # only does the x0*S materialize.
                # The psum ends as umix + S_l (eye matmul); DVE then
                # materializes x_{l+1} = (psum + bias) * x0 in ONE op
                # (scalar_tensor_tensor), skipping the ACT hop on the
                # critical path.  ACT still copies S_new = psum + bias to
                # SBUF, but only when layer l+1 needs it for its eye
                # matmul (l < LAYERS-1) -- off the critical path.
                last = l == LAYERS - 1
                if not last:
                    S_new = work.tile([128, KC, NB], F32R, tag="S", bufs=2,
                                      name=f"S{b}_{l}")
                if last:
                    tgt = work.tile([128, KC, NB], F32, tag="xl", name=f"osb{b}")
                else:
                    tgt = work.tile([128, KC, NB], F32R, tag="xl", name=f"xl{b}_{l}")
                for m in range(KC):
                    ups = pp.tile([128, NB], F32, tag="ups", bufs=3, name=f"ups{b}_{l}_{m}")
                    for kc in range(2):
                        nc.tensor.matmul(
                            ups,
                            (ua_sb[l][kc][:, m * 128:(m + 1) * 128]),
                            (cg[kc]),
                            start=(kc == 0), stop=(kc == 1 and l == 0))
                    if l > 0:
                        nc.tensor.matmul(ups, eye_sb, S_prev[:, m, :],
                                         start=False, stop=True)
                    bcol = bias_sb[l][:, m:m + 1]
                    nc.vector.scalar_tensor_tensor(
                        out=tgt[:, m, :], in0=ups, scalar=bcol,
                        in1=x0[:, m, :].bitcast(F32),
                        op0=OP.add, op1=OP.mult)
                    if not last:
                        nc.scalar.activation(S_new[:, m, :], ups, AF.Identity,
                                             bias=bcol)
                    else:
                        nc.sync.dma_start(
                            out=outT_r[:, m, b * NB:(b + 1) * NB],
                            in_=tgt[:, m, :])
                if not last:
                    S_prev = S_new
                    xl = tgt


def build_bass(n_blocks=BC // NB):
    nc = bacc.Bacc(trn_type="TRN2", target_bir_lowering=False, debug=False)
    bc = n_blocks * NB

    def inp(name, shape, dt=F32R):
        return nc.dram_tensor(name, list(shape), dt, kind="ExternalInput").ap()

    xT = inp("xT", (D, bc))
    w1 = inp("w1", (LAYERS, D, ER))
    gt = inp("gt", (D, E))
    cw = inp("cw", (LAYERS, 2, 128, 128))
    ua = inp("ua", (LAYERS, ER, D))
    biasP = inp("biasP", (LAYERS, D), F32)
    sel = inp("sel", (E, ER))
    onesE = inp("onesE", (E, E))
    eye = inp("eye", (128, 128))
    outT = nc.dram_tensor("outT", [D, bc], F32, kind="ExternalOutput").ap()

    with tile.TileContext(nc) as tc:
        _emit(tc, outT, xT, w1, gt, cw, ua, biasP, sel, onesE, eye, n_blocks)
    nc.compile()
    return nc


def prep_weights(U, V, C, bias, G):
    """Host-side weight rearrangement (replicated across cores)."""
    U = np.asarray(U, np.float32)
    V = np.asarray(V, np.float32)
    C = np.asarray(C, np.float32)
    bias = np.asarray(bias, np.float32)
    G = np.asarray(G, np.float32)

    # w1[i, d, e*R+r] = V[i, e, d, r]
    w1 = np.ascontiguousarray(V.transpose(0, 2, 1, 3).reshape(LAYERS, D, ER))
    # ua[i, e*R+r, d] = U[i, e, d, r]
    ua = np.ascontiguousarray(U.transpose(0, 1, 3, 2).reshape(LAYERS, ER, D))
    # block-diagonal C^T chunks: chunk j holds experts 2j, 2j+1
    cw = np.zeros((LAYERS, 2, 128, 128), np.float32)
    for i in range(LAYERS):
        for e in range(E):
            j, o = divmod(e, 2)
            cw[i, j, o * R:(o + 1) * R, o * R:(o + 1) * R] = C[i, e].T
    gt = np.ascontiguousarray(G.T)
    biasP = bias.copy()
    biasP[0] += 1.0  # S_0 = 1 folded into layer-0 bias
    sel = np.zeros((E, ER), np.float32)
    for e in range(E):
        sel[e, e * R:(e + 1) * R] = 1.0
    onesE = np.ones((E, E), np.float32)
    eye = np.eye(128, dtype=np.float32)
    return dict(w1=w1, gt=gt, cw=cw, ua=ua, biasP=biasP, sel=sel,
                onesE=onesE, eye=eye)


_NC_CACHE = {}


def _get_nc(n_blocks):
    if n_blocks not in _NC_CACHE:
        _NC_CACHE[n_blocks] = build_bass(n_blocks)
    return _NC_CACHE[n_blocks]


def run(inputs, trace=False, **spmd_kwargs):
    """Shard, run on 8 cores, gather.  Returns (output, BassKernelResults)."""
    x = np.asarray(inputs["x"], np.float32)
    weights = prep_weights(inputs["U"], inputs["V"], inputs["C"],
                           inputs["bias"], inputs["G"])
    nc = _get_nc(BC // NB)

    in_maps = []
    for c in range(NCORES):
        xc = np.ascontiguousarray(x[c * BC:(c + 1) * BC].T)  # (D, BC)
        in_maps.append(dict(xT=xc, **weights))

    res = run_bass_kernel_spmd(nc, in_maps, core_ids=list(range(NCORES)),
                               trace=trace, **spmd_kwargs)

    out = np.empty((B, D), np.float32)
    for c in range(NCORES):
        out[c * BC:(c + 1) * BC] = res.results[c]["outT"].T
    return out, res


def kernel(**inputs):
    out, _ = run(inputs)
    return out

